# revision 1
# baseline (speedup 1.0000x reference)
"""Trainium2 Bass kernel for nn_Connector (rmsnorm -> tiny matvec -> sinkhorn
-> per-token 4x4 mixing), data-parallel over 8 NeuronCores.

Self-contained: hardcodes all shapes; imports only the concourse/bass stack
that ships with the container.

Per-core layout (1024 tokens, 8 tiles of 128 tokens; tokens on partitions):
  - ms       : ScalarE Square with fused accum  -> sum(x^2) per token
  - rsqrt    : exp(-0.5*ln(ms/F + eps))  (single ACT table set: ln+exp)
  - G matvec : PE transposes 128x128 blocks -> Phi-chunk matmuls accumulate
               in PSUM (contracting the 8192 feature dim)
  - sinkhorn : 20 linear-space iterations on [128,16] tiles (VectorE),
               mathematically identical to the reference's log-space version
  - mixing   : out_i = sum_j diag(M[:,i,j]) @ res_j + diag(H_i) @ outp
               as PE matmuls with diagonal stationary operands (contraction
               over the token-partition axis), accumulated in PSUM
"""
import os
import sys

for _p in (
    "/opt/trn_rl_repo",
    "/opt/trn_rl_repo/pypackages",
    "/root/.axon_site/_ro/trn_rl_repo",
    "/root/.axon_site/_ro/pypackages",
):
    if os.path.isdir(_p) and _p not in sys.path:
        sys.path.append(_p)

from contextlib import ExitStack

import numpy as np

import concourse.bacc as bacc
import concourse.bass as bass
import concourse.tile as tile
from concourse import mybir
from concourse.bass_utils import run_bass_kernel_spmd

F32 = mybir.dt.float32
F32R = mybir.dt.float32r
BF16 = mybir.dt.bfloat16
AF = mybir.ActivationFunctionType
ALU = mybir.AluOpType
AX = mybir.AxisListType

# Problem constants
B, S, N, C = 4, 2048, 4, 2048
NCORES = 8
TOK = B * S                # 8192 tokens total
TPC = TOK // NCORES        # 1024 tokens per core
P = 128                    # tokens per tile (partition dim)
NTILES = TPC // P          # 8 tiles per core
F = N * C                  # 8192 features per token
NFB = F // P               # 64 feature blocks of 128
G20 = N + N * N            # 20 matvec outputs per token
EPS = 1e-5
ITERS = 20


def _kernel_body(ctx, tc, out_d, res_d, outp_d, phi_d, bias_d, eye_d):
    nc = tc.nc

    consts = ctx.enter_context(tc.tile_pool(name="consts", bufs=1))
    res_pool = ctx.enter_context(tc.tile_pool(name="res", bufs=3))
    outp_pool = ctx.enter_context(tc.tile_pool(name="outp", bufs=2))
    junk_pool = ctx.enter_context(tc.tile_pool(name="junk", bufs=1))
    tsb_pool = ctx.enter_context(tc.tile_pool(name="tsb", bufs=3))
    small_pool = ctx.enter_context(tc.tile_pool(name="small", bufs=2))
    diag_pool = ctx.enter_context(tc.tile_pool(name="diag", bufs=2))
    osb_pool = ctx.enter_context(tc.tile_pool(name="osb", bufs=3))

    tp_psum = ctx.enter_context(tc.tile_pool(name="tp_ps", bufs=2, space="PSUM"))
    g_psum = ctx.enter_context(tc.tile_pool(name="g_ps", bufs=1, space="PSUM"))
    gt_psum = ctx.enter_context(tc.tile_pool(name="gt_ps", bufs=1, space="PSUM"))
    mix_psum = ctx.enter_context(tc.tile_pool(name="mix_ps", bufs=2, space="PSUM"))

    # constants (f32r: PE matmuls run at 1 cycle/row instead of fp32's 4)
    phi_sb = consts.tile([P, NFB, G20], F32R)
    nc.sync.dma_start(phi_sb[:], phi_d.rearrange("(c p) m -> p c m", p=P))
    eye_sb = consts.tile([P, P], F32R)
    nc.sync.dma_start(eye_sb[:], eye_d[:])
    eye20 = consts.tile([G20, G20], F32)
    nc.vector.tensor_copy(eye20[:], eye_sb[0:G20, 0:G20].bitcast(F32))
    bias_sb = consts.tile([P, G20], F32)
    nc.sync.dma_start(bias_sb[:], bias_d[:].partition_broadcast(P))
    zero_sb = consts.tile([P, 1], F32)
    nc.vector.memset(zero_sb[:], 0.0)
    eps_sb = consts.tile([P, 1], F32)
    nc.vector.memset(eps_sb[:], EPS)

    for k in range(NTILES):
        tok = slice(k * P, (k + 1) * P)
        res_t = res_pool.tile([P, F], F32R)
        nc.sync.dma_start(res_t[:], res_d[tok, :])
        outp_t = outp_pool.tile([P, C], F32R)
        nc.sync.dma_start(outp_t[:], outp_d[tok, :])

        # ---- mean-square (ACT: square with fused accumulate) ----
        junk = junk_pool.tile([P, F], BF16)
        ssq = small_pool.tile([P, 1], F32)
        nc.scalar.activation(out=junk[:], in_=res_t[:].bitcast(F32),
                             func=AF.Square, bias=zero_sb[:],
                             accum_out=ssq[:])
        # rsq = exp(-0.5 * ln(ssq/F + eps))
        lnv = small_pool.tile([P, 1], F32)
        nc.scalar.activation(out=lnv[:], in_=ssq[:], func=AF.Ln,
                             scale=float(1.0 / F), bias=eps_sb[:])
        rsq = small_pool.tile([P, 1], F32)
        nc.scalar.activation(out=rsq[:], in_=lnv[:], func=AF.Exp, scale=-0.5,
                             bias=zero_sb[:])

        # ---- G = flat @ phi via PE transposes + accumulating matmuls ----
        # t_sb has a 128-col garbage tail so every G matmul can stream
        # N=256 columns (f32r needs moving dim >= 256 for full rate);
        # psum columns [128:256] accumulate junk and are never read.
        g_ps = g_psum.tile([G20, 2 * P], F32)
        for fq in range(NFB // 4):
            t_ps = tp_psum.tile([P, 512], F32R)
            for q in range(4):
                fb = fq * 4 + q
                nc.tensor.transpose(t_ps[:, q * P:(q + 1) * P],
                                    res_t[:, fb * P:(fb + 1) * P], eye_sb[:])
            t_sb = tsb_pool.tile([P, 640], F32R)
            nc.scalar.copy(out=t_sb[:, 0:512], in_=t_ps[:])
            for q in range(4):
                fb = fq * 4 + q
                nc.tensor.matmul(g_ps[:], phi_sb[:, fb, :],
                                 t_sb[:, q * P:q * P + 2 * P],
                                 start=(fb == 0), stop=(fb == NFB - 1))

        g_sb = small_pool.tile([G20, P], F32)
        nc.vector.tensor_copy(g_sb[:], g_ps[:, 0:P])
        gt_ps = gt_psum.tile([P, G20], F32)
        nc.tensor.transpose(gt_ps[:], g_sb[:], eye20[:])

        # tilde = G * rsq + bias
        tilde = small_pool.tile([P, G20], F32)
        nc.vector.tensor_scalar_mul(tilde[:], in0=gt_ps[:], scalar1=rsq[:])
        nc.vector.tensor_add(tilde[:], tilde[:], bias_sb[:])

        # ---- H = 2*sigmoid(tilde_post) = 2/(1+exp(-x)) ----
        hv = small_pool.tile([P, N], F32)
        nc.scalar.activation(out=hv[:], in_=tilde[:, 0:N], func=AF.Exp,
                             scale=-1.0, bias=zero_sb[:])
        nc.vector.tensor_scalar_add(hv[:], in0=hv[:], scalar1=1.0)
        nc.vector.reciprocal(hv[:], hv[:])
        nc.vector.tensor_scalar_mul(hv[:], in0=hv[:], scalar1=2.0)

        # ---- sinkhorn (linear space) ----
        m_sb = small_pool.tile([P, N * N], F32)
        nc.scalar.activation(out=m_sb[:], in_=tilde[:, N:G20], func=AF.Exp,
                             bias=zero_sb[:])
        m3 = m_sb[:].rearrange("p (i j) -> p i j", i=N)
        rs = small_pool.tile([P, N], F32)
        rr = small_pool.tile([P, N], F32)
        cs = small_pool.tile([P, N], F32)
        cr = small_pool.tile([P, N], F32)
        rr_b = rr[:].unsqueeze(2).broadcast_to([P, N, N])
        cr_b = cr[:].unsqueeze(1).broadcast_to([P, N, N])
        for _ in range(ITERS):
            nc.vector.tensor_reduce(out=rs[:], in_=m3, axis=AX.X, op=ALU.add)
            nc.vector.reciprocal(rr[:], rs[:])
            nc.vector.tensor_tensor(out=m3, in0=m3, in1=rr_b, op=ALU.mult)
            nc.vector.tensor_reduce(out=cs[:], in_=m3.transpose([0, 2, 1]),
                                    axis=AX.X, op=ALU.add)
            nc.vector.reciprocal(cr[:], cs[:])
            nc.vector.tensor_tensor(out=m3, in0=m3, in1=cr_b, op=ALU.mult)

        # ---- build diagonal stationary operands ----
        diag = diag_pool.tile([P, G20, P], F32R)
        eye_f = eye_sb[:].bitcast(F32)
        for idx in range(N * N):
            nc.vector.tensor_scalar_mul(diag[:, idx, :], in0=eye_f,
                                        scalar1=m_sb[:, idx:idx + 1])
        for i in range(N):
            nc.vector.tensor_scalar_mul(diag[:, N * N + i, :], in0=eye_f,
                                        scalar1=hv[:, i:i + 1])

        # ---- mixing: out_i = sum_j diag(M_ij) @ res_j + diag(H_i) @ outp ----
        for i in range(N):
            for half in range(2):
                mix_ps = mix_psum.tile([P, 1024], F32)
                for c2 in range(2):
                    seg = slice(c2 * 512, (c2 + 1) * 512)
                    c0 = half * 1024 + c2 * 512
                    for j in range(N):
                        nc.tensor.matmul(mix_ps[:, seg],
                                         diag[:, i * N + j, :],
                                         res_t[:, j * C + c0: j * C + c0 + 512],
                                         start=(j == 0), stop=False)
                    nc.tensor.matmul(mix_ps[:, seg],
                                     diag[:, N * N + i, :],
                                     outp_t[:, c0:c0 + 512],
                                     start=False, stop=True)
                o_sb = osb_pool.tile([P, 1024], F32)
                if half == 0:
                    nc.vector.tensor_copy(o_sb[:], mix_ps[:])
                else:
                    nc.scalar.copy(out=o_sb[:], in_=mix_ps[:])
                nc.sync.dma_start(
                    out_d[tok, i * C + half * 1024: i * C + half * 1024 + 1024],
                    o_sb[:])


def build_nc():
    nc = bacc.Bacc("TRN2", target_bir_lowering=False)
    res_d = nc.declare_dram_parameter("residual", [TPC, F], F32R, isOutput=False)
    outp_d = nc.declare_dram_parameter("outp", [TPC, C], F32R, isOutput=False)
    phi_d = nc.declare_dram_parameter("phi", [F, G20], F32R, isOutput=False)
    bias_d = nc.declare_dram_parameter("bias", [G20], F32, isOutput=False)
    eye_d = nc.declare_dram_parameter("eye", [P, P], F32R, isOutput=False)
    out_d = nc.declare_dram_parameter("out", [TPC, F], F32, isOutput=True)
    with tile.TileContext(nc) as tc, ExitStack() as ctx:
        _kernel_body(ctx, tc, out_d[:], res_d[:], outp_d[:], phi_d[:],
                     bias_d[:], eye_d[:])
    if not nc.is_finalized():
        nc.finalize()
    return nc


_NC_CACHE = {}


def _get_nc():
    if "nc" not in _NC_CACHE:
        _NC_CACHE["nc"] = build_nc()
    return _NC_CACHE["nc"]


def _prep_in_maps(residual, output, rms_scale, phi_post, phi_res, b_post,
                  b_res, alpha_post, alpha_res):
    residual = np.ascontiguousarray(np.asarray(residual, dtype=np.float32))
    output = np.ascontiguousarray(np.asarray(output, dtype=np.float32))
    rms_scale = np.asarray(rms_scale, dtype=np.float32)
    phi_post = np.asarray(phi_post, dtype=np.float32)
    phi_res = np.asarray(phi_res, dtype=np.float32)
    b_post = np.asarray(b_post, dtype=np.float32)
    b_res = np.asarray(b_res, dtype=np.float32)
    a_post = float(np.asarray(alpha_post))
    a_res = float(np.asarray(alpha_res))

    phi_cat = np.ascontiguousarray(
        np.concatenate([a_post * phi_post, a_res * phi_res], axis=1)
        * rms_scale[:, None]).astype(np.float32)
    bias_cat = np.concatenate([b_post, b_res.reshape(-1)]).astype(np.float32)
    eye = np.eye(P, dtype=np.float32)

    res_flat = residual.reshape(TOK, F)
    outp_flat = output.reshape(TOK, C)
    in_maps = []
    for c in range(NCORES):
        sl = slice(c * TPC, (c + 1) * TPC)
        in_maps.append({
            "residual": np.ascontiguousarray(res_flat[sl]),
            "outp": np.ascontiguousarray(outp_flat[sl]),
            "phi": phi_cat,
            "bias": bias_cat,
            "eye": eye,
        })
    return in_maps


def run_sharded(trace=False, **inputs):
    """Run on hardware; returns (full_output, exec_time_ns)."""
    in_maps = _prep_in_maps(**inputs)
    nc = _get_nc()
    r = run_bass_kernel_spmd(nc, in_maps, list(range(NCORES)), trace=trace)
    outs = [np.asarray(r.results[c]["out"]) for c in range(NCORES)]
    full = np.concatenate(outs, axis=0).reshape(B, S, N, C).astype(np.float32)
    return full, r.exec_time_ns


def kernel(**inputs):
    full, _ = run_sharded(trace=False, **inputs)
    return full

